# revision 16
# baseline (speedup 1.0000x reference)
"""BitFeedForward Trainium2 kernel (BitNet b1.58 FFN: act-quant -> w1 -> gelu
-> act-quant -> w2), data-parallel over tokens across the NeuronCores.

Math notes (same arithmetic path as v3, proven on HW):
- activation_quant: q = round(x * s), s = 127/clip(rowmax|x|,1e-5). |q|<=127 so
  quantized values are exactly representable in bf16; the matmul of int-valued
  bf16 against ternary bf16 accumulated in fp32 PSUM is EXACT.
- weight_quant: tern = clip(round(w*s1), -1, 1), s1 = 1/clip(mean|w|,1e-5),
  computed as round(clamp(w*s1, +-1.49999988)) via the fp32 magic-number trick.
- mean|w|: computed on the HOST with jax-on-CPU so it matches the reference's
  jnp.mean bitwise, passed as the tiny "wm" input.

v5 changes (same arithmetic as v3/v4, fewer+bigger instructions):
- phase-1 H-chunk and phase-2 token-chunk are 1024 (2 PSUM banks per matmul):
  half the matmul/Ldweights/DMA instruction count of v3.
- x-quant transposes via XBAR (dma_start_transpose on int-valued bf16)
  instead of 512 PE transposes + PSUM round-trips.
- host-blocked weight layouts (w1p, w2p): every weight DMA is [128, 4096] f32
  with 16KB contiguous per partition.
- no w2 DRAM bf16 cache (v3 spent a 32MB roundtrip on it); phase 2
  inline-ternarizes all 16 d-tiles.
- output written TRANSPOSED (outT[d, t]); host untransposes.
"""

from contextlib import ExitStack

import numpy as np

import concourse.bacc as bacc
import concourse.tile as tile
from concourse import mybir
from concourse.masks import make_identity

F32 = mybir.dt.float32
BF16 = mybir.dt.bfloat16
F16 = mybir.dt.float16
AX = mybir.AxisListType
OP = mybir.AluOpType
AF = mybir.ActivationFunctionType

MAGIC = 1.5 * 2**23  # fp32 round-to-nearest-even magic constant
CLIP = 1.49999988    # largest fp32 < 1.5
EPS = 1e-5
INV127 = 1.0 / 127.0


def build_kernel(T, D, H, n_cores, reps=1, do_phase1=True, do_phase2=True):
    """Build the per-core SPMD kernel.

    Per-core inputs: x [T,D],
    w1p [128, H*D/128]  (w1p[p, (hc*Dk+k)*HC + j] = w1[hc*HC+j, k*128+p]),
    w2p [128, D*H/128]  (w2p[p, (dd*Hk+s)*128 + d] = w2[dd*128+d, s*128+p]),
    wm [1,4] host scales.  Output: out [D, T] (transposed; host untransposes).
    """
    Tt = T // 128          # token tiles (8)
    Dk = D // 128          # k-tiles of D (phase-1 contraction, 16)
    HC = 1024              # phase-1 H chunk (two PSUM banks of f32)
    NC1 = H // HC          # 8
    Hk = H // 128          # H k-tiles (phase-2 contraction, 64)
    TC = 512               # phase-2 token chunk (one PSUM bank)
    NTC = T // TC          # 2
    ND2 = D // 128         # phase-2 d tiles (16)
    HP = 2048              # phase-1.5 h piece width
    NHP = H // HP          # 4
    W2H = Hk // 4          # phase-2 w2 quarter (s-tiles, 16)

    nc = bacc.Bacc("TRN2", target_bir_lowering=False, debug=False,
                   num_devices=n_cores)

    x_ap = nc.dram_tensor("x", [T, D], F32, kind="ExternalInput").ap()
    w1p_ap = nc.dram_tensor("w1p", [128, H * D // 128], F32,
                            kind="ExternalInput").ap()
    w2p_ap = nc.dram_tensor("w2p", [128, D * H // 128], F32,
                            kind="ExternalInput").ap()
    # host-computed weight-scale vector [1/m1, 1/m2, m1, m2]
    wm_ap = nc.dram_tensor("wm", [1, 4], F32, kind="ExternalInput").ap()
    out_ap = nc.dram_tensor("out", [D, T], F32, kind="ExternalOutput").ap()

    with tile.TileContext(nc) as tc:
        with ExitStack() as ctx:
            persist = ctx.enter_context(tc.tile_pool(name="persist", bufs=1))
            stage = ctx.enter_context(tc.tile_pool(name="stage", bufs=1))
            dram = ctx.enter_context(
                tc.tile_pool(name="dram", bufs=1, space="DRAM"))
            psum = ctx.enter_context(
                tc.tile_pool(name="psum", bufs=1, space="PSUM"))

            def ps_mm():
                return psum.tile([128, 512], F32, tag="ps", name="ps",
                                 bufs=8)

            def stsm(nm):
                return stage.tile([128, 1], F32, tag="stsm", name=nm, bufs=4)

            # ---- constants ----
            magicv = persist.tile([128, 1], F32, tag="magicv")
            nc.gpsimd.memset(magicv[:], MAGIC)
            ident = persist.tile([128, 128], F32, tag="ident")
            make_identity(nc, ident[:])
            ones_row = persist.tile([1, 128], F32, tag="ones_row")
            nc.gpsimd.memset(ones_row[:], 1.0)

            # cols per t: 0=sx 1=invsx 2=deq1 3=runmax 4=sh 5=invsh 6=deq2
            pertok = persist.tile([128, 8 * Tt], F32, tag="pertok")
            vals = persist.tile([1, 4], F32, tag="vals")
            bcast = persist.tile([128, 4], F32, tag="bcast")
            d2r = persist.tile([1, T], F32, tag="d2r")
            Bd2 = persist.tile([128, T], F32, tag="Bd2")

            hbuf = dram.tile([T, H], F16, tag="hbuf")

            S1 = bcast[:, 0:1]
            S2 = bcast[:, 1:2]
            M2W = bcast[:, 3:4]

            def one_pass():
                with ExitStack() as ctxA:
                    pool_a = ctxA.enter_context(
                        tc.tile_pool(name="pool_a", bufs=1))
                    xqt = pool_a.tile([128, Tt * Dk * 128], BF16, tag="xqt")
                    xqt_v = xqt[:].rearrange("p (a c) -> p a c", c=128)

                    # ---- phase 0a: x load/quant, XBAR transpose to xqt.
                    # x loads ride the scalar queue (shared later with hbuf
                    # stores, which only start after these loads are done).
                    for t in range(Tt):
                        xt = pool_a.tile([128, 2048], F32, tag="st8a",
                                         name="xt", bufs=2)
                        nc.scalar.dma_start(xt[:, 0:D],
                                            x_ap[t * 128:(t + 1) * 128, :])
                        sx = pertok[:, 8 * t + 0:8 * t + 1]
                        invsx = pertok[:, 8 * t + 1:8 * t + 2]
                        mx = stsm("mx")
                        nc.vector.tensor_reduce(mx[:], xt[:, 0:D], axis=AX.X,
                                                op=OP.max,
                                                apply_absolute_value=True)
                        nc.vector.tensor_scalar(invsx, mx[:], EPS, INV127,
                                                OP.max, OP.mult)
                        nc.vector.reciprocal(sx, invsx)
                        qx = pool_a.tile([128, 2048], F32, tag="st8a",
                                         name="qx", bufs=2)
                        nc.scalar.activation(qx[:, 0:D], xt[:, 0:D],
                                             AF.Identity,
                                             bias=magicv[:, 0:1], scale=sx)
                        qxb = pool_a.tile([128, 2048], BF16, tag="qxb",
                                          name="qxb", bufs=2)
                        nc.vector.tensor_scalar(qxb[:, 0:D], qx[:, 0:D],
                                                MAGIC, None, OP.subtract)
                        # XBAR: xqt[:, t*Dk + k, :] = qxb[:, k*128:...].T
                        nc.scalar.dma_start_transpose(
                            xqt_v[:, t * Dk:(t + 1) * Dk, :], qxb[:, 0:D])

                    # ---- phase 0b: host-computed weight scales ----
                    nc.sync.dma_start(vals[:, 0:4], wm_ap[:])
                    psb = ps_mm()
                    nc.tensor.matmul(psb[:, 0:4], ones_row[:],
                                     vals[:, 0:4], start=True, stop=True)
                    nc.scalar.copy(bcast[:], psb[:, 0:4])

                    # deq1_t = m1w * invsx_t
                    for t in range(Tt):
                        nc.vector.tensor_scalar(pertok[:, 8 * t + 2:8 * t + 3],
                                                pertok[:, 8 * t + 1:8 * t + 2],
                                                bcast[:, 2:3], None, OP.mult)

                    # ---- phase 1: h = gelu(deq1 * (xq @ w1q^T)), rowmax ----
                    # w1 loads on the sync queue only; hbuf stores on scalar.
                    for hc in (range(NC1) if do_phase1 else ()):
                        w1q = pool_a.tile([128, Dk, HC], BF16, tag="wq",
                                          name="w1q", bufs=2)
                        for wh in range(4):
                            kh = Dk // 4
                            w1f = pool_a.tile([128, kh * HC], F32, tag="wf",
                                              name="w1f", bufs=2)
                            base = (hc * Dk + wh * kh) * HC
                            nc.sync.dma_start(
                                w1f[:], w1p_ap[:, base:base + kh * HC])
                            nc.gpsimd.tensor_scalar(w1f[:], w1f[:], S1, CLIP,
                                                    OP.mult, OP.min)
                            nc.vector.tensor_scalar(w1f[:], w1f[:], -CLIP,
                                                    MAGIC, OP.max, OP.add)
                            nc.scalar.activation(
                                w1q[:, wh * kh:(wh + 1) * kh, :]
                                .rearrange("p a b -> p (a b)"),
                                w1f[:], AF.Copy, bias=-MAGIC)
                        for t in range(Tt):
                            hsb = pool_a.tile([128, HC], F32, tag="hsb",
                                              name="hsb", bufs=3)
                            for hh in range(2):
                                ps = ps_mm()
                                for k in range(Dk):
                                    nc.tensor.matmul(
                                        ps[:, 0:512],
                                        xqt[:, (t * Dk + k) * 128:
                                            (t * Dk + k) * 128 + 128],
                                        w1q[:, k, hh * 512:(hh + 1) * 512],
                                        start=(k == 0), stop=(k == Dk - 1))
                                nc.scalar.activation(
                                    hsb[:, hh * 512:(hh + 1) * 512],
                                    ps[:, 0:512], AF.Gelu,
                                    scale=pertok[:, 8 * t + 2:8 * t + 3])
                            mx1 = stsm("mx1")
                            nc.vector.tensor_reduce(
                                mx1[:], hsb[:], axis=AX.X, op=OP.max,
                                apply_absolute_value=True)
                            runmax = pertok[:, 8 * t + 3:8 * t + 4]
                            if hc == 0:
                                nc.vector.tensor_copy(runmax, mx1[:])
                            else:
                                nc.vector.tensor_max(runmax, runmax, mx1[:])
                            # store h as f16 (rowmax was taken from f32, so
                            # the quant scale matches the reference bitwise;
                            # the f16 value round only perturbs q by <=1 for
                            # products within 2^-11 of a .5 boundary)
                            hb = pool_a.tile([128, HC], F16, tag="hb",
                                             name="hb", bufs=2)
                            nc.vector.tensor_copy(hb[:], hsb[:])
                            nc.scalar.dma_start(
                                hbuf[t * 128:(t + 1) * 128,
                                     hc * HC:(hc + 1) * HC], hb[:])

                if not do_phase2:
                    zt = stage.tile([128, 512], F32, tag="zt", bufs=1)
                    nc.gpsimd.memset(zt[:], 0.0)
                    nc.scalar.dma_start(out_ap[0:128, 0:512], zt[:])
                    return

                # ---- phases 1.5 + 2 share one scope (overlap enabled) ----
                with ExitStack() as ctxB:
                    pool_b = ctxB.enter_context(
                        tc.tile_pool(name="pool_b", bufs=1))
                    # hqT, SBUF-resident: block s at cols [s*T, (s+1)*T),
                    # within a block token-tile t at [t*128, (t+1)*128).
                    hqt = pool_b.tile([128, Hk * T], BF16, tag="hqt")
                    hqt_v = hqt[:].rearrange("p (s c) -> p s c", c=T)

                    # per-token h scales
                    for t in range(Tt):
                        sh = pertok[:, 8 * t + 4:8 * t + 5]
                        invsh = pertok[:, 8 * t + 5:8 * t + 6]
                        deq2 = pertok[:, 8 * t + 6:8 * t + 7]
                        runmax = pertok[:, 8 * t + 3:8 * t + 4]
                        nc.vector.tensor_scalar(invsh, runmax, EPS, INV127,
                                                OP.max, OP.mult)
                        nc.vector.reciprocal(sh, invsh)
                        nc.vector.tensor_scalar(deq2, invsh, M2W, None,
                                                OP.mult)

                    # Bd2[128, T]: column j*128+i = deq2[token-tile j, tok i].
                    # Built mid-way through the first phase-1.5 round so the
                    # PE broadcast matmuls hide inside the h-piece DMA window
                    # (and before pscs_i claims every PSUM bank).
                    def build_bd2():
                        for half in range(T // 512):
                            psd = ps_mm()
                            for t4 in range(4):
                                t = half * 4 + t4
                                nc.tensor.matmul(
                                    psd[0:1, t4 * 128:(t4 + 1) * 128],
                                    pertok[:, 8 * t + 6:8 * t + 7], ident[:],
                                    start=True, stop=True)
                            nc.scalar.copy(
                                d2r[0:1, half * 512:(half + 1) * 512],
                                psd[0:1, 0:512])
                        for half in range(T // 512):
                            psb2 = ps_mm()
                            nc.tensor.matmul(
                                psb2[:, 0:512], ones_row[:],
                                d2r[0:1, half * 512:(half + 1) * 512],
                                start=True, stop=True)
                            nc.scalar.copy(
                                Bd2[:, half * 512:(half + 1) * 512],
                                psb2[:, 0:512])

                    # helpers for phase 2
                    def w2_quarter(dd, q):
                        """Load + ternarize w2q s-tiles [q*W2H,(q+1)*W2H) for
                        output d-tile dd from the blocked w2p layout: one
                        contiguous [128, 2048] f32 DMA."""
                        w2f = pool_b.tile([128, W2H * 128], F32, tag="w2f",
                                          name="w2f", bufs=2)
                        base = (dd * Hk + q * W2H) * 128
                        nc.sync.dma_start(
                            w2f[:], w2p_ap[:, base:base + W2H * 128])
                        nc.gpsimd.tensor_scalar(w2f[:], w2f[:], S2, CLIP,
                                                OP.mult, OP.min)
                        nc.vector.tensor_scalar(w2f[:], w2f[:], -CLIP,
                                                MAGIC, OP.max, OP.add)
                        w2q = pool_b.tile([128, W2H, 128], BF16, tag="w2q",
                                          name="w2q", bufs=4)
                        nc.scalar.activation(
                            w2q[:].rearrange("p a b -> p (a b)"),
                            w2f[:], AF.Copy, bias=-MAGIC)
                        return w2q

                    def mm_block(pscs, w2q, q, cs=None):
                        for sl in range(W2H):
                            s = q * W2H + sl
                            for c in (range(NTC) if cs is None else cs):
                                nc.tensor.matmul(
                                    pscs[c][:, 0:TC],
                                    w2q[:, sl, :],
                                    hqt[:, s * T + c * TC:
                                        s * T + (c + 1) * TC],
                                    start=(s == 0), stop=(s == Hk - 1),
                                    skip_group_check=True)

                    def dd_epilogue(dd, pscs):
                        osb = stage.tile([128, T], F32, tag="osb",
                                         name="osb", bufs=1)
                        for c in range(NTC):
                            nc.vector.tensor_tensor(
                                osb[:, c * TC:(c + 1) * TC],
                                pscs[c][:, 0:TC],
                                Bd2[:, c * TC:(c + 1) * TC], OP.mult)
                        nc.scalar.dma_start(
                            out_ap[dd * 128:(dd + 1) * 128, :], osb[:])

                    # phase 1.5: read h back, quantize, XBAR-transpose into
                    # hqt -- piece-outer so finished s-ranges feed the first
                    # NDI dd-chains of phase 2 while later pieces stream.
                    NDI = 4
                    assert HP // 128 == W2H

                    def h_piece(t, p):
                        sh = pertok[:, 8 * t + 4:8 * t + 5]
                        hrd = pool_b.tile([128, HP], F16, tag="hrd",
                                          name="hrd", bufs=2)
                        nc.sync.dma_start(
                            hrd[:], hbuf[t * 128:(t + 1) * 128,
                                         p * HP:(p + 1) * HP])
                        # f32 RNE round via magic constant: tmp = h*sh + MAGIC
                        tmp = pool_b.tile([128, HP], F32, tag="hr32",
                                          name="hr32", bufs=2)
                        nc.scalar.activation(tmp[:], hrd[:], AF.Identity,
                                             bias=magicv[:, 0:1], scale=sh)
                        hqp = stage.tile([128, HP], BF16, tag="hqp",
                                         name="hqp", bufs=2)
                        nc.vector.tensor_scalar(hqp[:], tmp[:],
                                                MAGIC, None, OP.subtract)
                        # batched 128x128 block transposes via XBAR:
                        # out[:, j, :] = hqp[:, j*128:(j+1)*128].T
                        dst3 = hqt_v[:, p * W2H:(p + 1) * W2H,
                                     t * 128:(t + 1) * 128]
                        nc.scalar.dma_start_transpose(dst3, hqp[:])

                    # p=0 round unrolled: Bd2 build + psum claims happen
                    # behind the first h-piece DMAs
                    w2qs0 = [w2_quarter(dd, 0) for dd in range(NDI)]
                    for t in range(Tt // 2):
                        h_piece(t, 0)
                    build_bd2()
                    pscs_i = [[ps_mm() for _ in range(NTC)]
                              for _ in range(NDI)]
                    for dd in range(NDI):
                        mm_block(pscs_i[dd], w2qs0[dd], 0, cs=(0,))
                    for t in range(Tt // 2, Tt):
                        h_piece(t, 0)
                    for dd in range(NDI):
                        mm_block(pscs_i[dd], w2qs0[dd], 0, cs=(1,))
                    for p in range(1, NHP):
                        w2qs = [w2_quarter(dd, p) for dd in range(NDI)]
                        for t in range(Tt // 2):
                            h_piece(t, p)
                        for dd in range(NDI):
                            mm_block(pscs_i[dd], w2qs[dd], p, cs=(0,))
                        for t in range(Tt // 2, Tt):
                            h_piece(t, p)
                        for dd in range(NDI):
                            mm_block(pscs_i[dd], w2qs[dd], p, cs=(1,))
                    for dd in range(NDI):
                        dd_epilogue(dd, pscs_i[dd])

                    # phase 2 remainder
                    for dd in range(NDI, ND2):
                        pscs = [ps_mm() for _ in range(NTC)]
                        for q in range(Hk // W2H):
                            w2q = w2_quarter(dd, q)
                            mm_block(pscs, w2q, q)
                        dd_epilogue(dd, pscs)

            for _rep in range(reps):
                one_pass()

    nc.compile()
    return nc


def shard_inputs(x, w1, w2, n_cores):
    """Host-side sharding: token shards + blocked weight layouts + scales."""
    B, S, Dx = x.shape
    T_total = B * S
    T = T_total // n_cores
    xf = np.ascontiguousarray(x.reshape(T_total, Dx))
    H = w1.shape[0]
    D = Dx
    HC = 1024
    Dk, NC1, Hk, ND2 = D // 128, H // HC, H // 128, D // 128
    # w1p[p, (hc*Dk+k)*HC + j] = w1[hc*HC+j, k*128+p]
    w1p = np.ascontiguousarray(
        w1.reshape(NC1, HC, Dk, 128).transpose(3, 0, 2, 1)
        .reshape(128, NC1 * Dk * HC))
    # w2p[p, (dd*Hk+s)*128 + d] = w2[dd*128+d, s*128+p]
    w2p = np.ascontiguousarray(
        w2.reshape(ND2, 128, Hk, 128).transpose(3, 0, 2, 1)
        .reshape(128, ND2 * Hk * 128))
    # mean|w| via jax-on-CPU so it matches the reference's jnp.mean bitwise
    try:
        import jax
        import jax.numpy as jnp
        with jax.default_device(jax.devices("cpu")[0]):
            m1 = np.float32(jnp.clip(jnp.mean(jnp.abs(jnp.asarray(w1))),
                                     np.float32(EPS), None))
            m2 = np.float32(jnp.clip(jnp.mean(jnp.abs(jnp.asarray(w2))),
                                     np.float32(EPS), None))
    except Exception:
        m1 = np.maximum(np.mean(np.abs(w1)).astype(np.float32),
                        np.float32(EPS))
        m2 = np.maximum(np.mean(np.abs(w2)).astype(np.float32),
                        np.float32(EPS))
    wm = np.array([[np.float32(1.0) / m1, np.float32(1.0) / m2, m1, m2]],
                  dtype=np.float32)
    in_maps = []
    for i in range(n_cores):
        in_maps.append({
            "x": np.ascontiguousarray(xf[i * T:(i + 1) * T]),
            "w1p": w1p,
            "w2p": w2p,
            "wm": wm,
        })
    return in_maps, (B, S, D, H, T)


# ---------------------------------------------------------------------------
# Self-contained entry point for grading: kernel(**inputs) -> np.ndarray
# ---------------------------------------------------------------------------
from concourse.bass_utils import run_bass_kernel_spmd

N_CORES = 8
B_, S_, D_, H_ = 4, 2048, 2048, 8192
T_ = (B_ * S_) // N_CORES  # tokens per core

_NC_CACHE = {}


def _get_nc():
    key = (T_, D_, H_, N_CORES)
    if key not in _NC_CACHE:
        _NC_CACHE[key] = build_kernel(T_, D_, H_, N_CORES)
    return _NC_CACHE[key]


def run_spmd(x, w1, w2, **run_kwargs):
    """Shard, run on the 8 cores, gather. Returns (out, BassKernelResults)."""
    x = np.asarray(x, dtype=np.float32)
    w1 = np.asarray(w1, dtype=np.float32)
    w2 = np.asarray(w2, dtype=np.float32)
    B, S, D = x.shape
    nc = _get_nc()
    in_maps, _meta = shard_inputs(x, w1, w2, N_CORES)
    res = run_bass_kernel_spmd(nc, in_maps, list(range(N_CORES)), **run_kwargs)
    # per-core outputs are [D, T]; transpose+concat on host
    outs = [np.asarray(res.results[i]["out"]).T for i in range(N_CORES)]
    out = np.concatenate(outs, axis=0).reshape(B, S, D).astype(np.float32)
    return out, res


def kernel(x, w1, w2):
    out, _ = run_spmd(x, w1, w2)
    return out
